# revision 1
# baseline (speedup 1.0000x reference)
"""MoE (top-2 of 8 experts, SwiGLU) Trainium2 kernel.

Strategy (expert-parallel over 8 NeuronCores):
  * Host: router GEMM + top-2 + sigmoid gates in numpy (selection verified to
    match the jax fp32 reference on these inputs), then gather each expert's
    tokens into a transposed, capacity-padded buffer xT_e [H, C]. One expert
    per core.
  * Device (SPMD, per core): two phases.
      Phase 1: h = silu(x @ Wg) * (x @ Wu), Wg/Wu SBUF-resident, h spilled
               to a DRAM scratch buffer (layout [I, C]).
      Phase 2: yT = (h @ Wd) * gate, Wd SBUF-resident, gate applied during
               PSUM eviction (out column t scaled by gate[t]).
    Matmuls run in float32r — IEEE fp32 layout with the mantissa rounded to
    11 bits (low 12 bits zero), which streams at full PE rate (1 cycle/row
    for moving dim >= 256) vs 4 cycles/row for fp32. Inputs are pre-rounded
    on the host (round-to-nearest-even bit trick); the h intermediate is
    rounded on-chip for free by giving the DVE multiply an fp32r output.
    Tokens are the moving dimension (512 wide), weights the 128x128
    stationary operand.
  * Host: out[idx_e] += yT_e[:, :n_e].T  (indices within one expert are
    unique, so fancy-index += is safe).
"""

import os
import numpy as np

T, H, I, E, TOPK = 8192, 1024, 2048, 8, 2
NCORES = 8
PB = 128

_compiled = {}
last_results = None  # BassKernelResults of the most recent run (for test harness)


def round_fp32r(a):
    """Round fp32 array to fp32r (11-bit mantissa, RNE), keeping fp32 layout."""
    u = np.ascontiguousarray(a, dtype=np.float32).view(np.uint32)
    r = (u + np.uint32(0x7FF) + ((u >> np.uint32(12)) & np.uint32(1))) \
        & np.uint32(0xFFFFF000)
    return r.view(np.float32)


def _tsegs(C):
    """Split C into segments of width 256..512 (fp32r full rate needs >=256)."""
    widths = []
    rem = C
    while rem >= 768:
        widths.append(512)
        rem -= 512
    if rem <= 512:
        widths.append(rem)
    else:
        widths.append(rem - 256)
        widths.append(256)
    segs = []
    t0 = 0
    for tb in widths:
        segs.append((t0, tb))
        t0 += tb
    return segs


def _build(C):
    import concourse.bacc as bacc
    import concourse.mybir as mybir
    import concourse.tile as tile

    fp32 = mybir.dt.float32
    fp32r = mybir.dt.float32r
    AF = mybir.ActivationFunctionType

    KB = H // PB   # 8 contraction blocks over H
    IB = I // PB   # 16 blocks over I
    HB = H // PB   # 8 output blocks over H

    nc = bacc.Bacc("TRN2", target_bir_lowering=False, debug=False,
                   num_devices=NCORES)
    xT = nc.dram_tensor("xT", [H, C], fp32r, kind="ExternalInput").ap()
    gm = nc.dram_tensor("gm", [PB, C], fp32, kind="ExternalInput").ap()
    Wg = nc.dram_tensor("Wg", [H, I], fp32r, kind="ExternalInput").ap()
    Wu = nc.dram_tensor("Wu", [H, I], fp32r, kind="ExternalInput").ap()
    Wd = nc.dram_tensor("Wd", [I, H], fp32r, kind="ExternalInput").ap()
    yT = nc.dram_tensor("yT", [H, C], fp32, kind="ExternalOutput").ap()
    hsp = nc.dram_tensor("hsp", [I, C], fp32r, kind="Internal").ap()

    segs = _tsegs(C)

    QW = 512           # weight-column quarter width
    NQ = I // QW       # 4 quarters
    IPQ = QW // PB     # 4 i-blocks per quarter

    # Split token segments into pair-groups so per-group x tiles fit in SBUF
    # while the quarter loop runs outermost (weights stream exactly once).
    halves = [segs[i:i + 2] for i in range(0, len(segs), 2)]
    N_EARLY = 4 if len(segs) > 1 else 0   # Wd tiles preloaded during phase 1

    from contextlib import ExitStack
    with tile.TileContext(nc) as tc, ExitStack() as _stack:
        wde = _stack.enter_context(tc.tile_pool(name="wde", bufs=1, side="right"))
        # Phase 1: h = silu(x@Wg) * (x@Wu) -> DRAM spill (fp32r)
        with tc.tile_pool(name="w1", bufs=1) as w1, \
             tc.tile_pool(name="xp", bufs=1) as xp, \
             tc.tile_pool(name="ev1", bufs=2) as ev1, \
             tc.tile_pool(name="ps1", bufs=3, space="PSUM") as ps1:
            wg_s = [[None] * NQ for _ in range(KB)]
            wu_s = [[None] * NQ for _ in range(KB)]

            def load_xt(si, t0, tb):
                tiles = []
                for k in range(KB):
                    xtk = xp.tile([PB, tb], fp32r, tag=f"xt{k}_{si}",
                                  name=f"xt{k}_{si}")
                    nc.sync.dma_start(
                        out=xtk[:], in_=xT[k * PB:(k + 1) * PB, t0:t0 + tb])
                    tiles.append(xtk)
                return tiles

            # q0 weight tiles live in their own pool, closed after their last
            # use so the freed SBUF can preload Wd tiles before phase 2.
            w1q0_stack = ExitStack()
            w1q0 = w1q0_stack.enter_context(tc.tile_pool(name="w1q0", bufs=1))

            # Interleave the first x tiles with the q0 gate weights so the
            # first matmul can issue after ~0.5MB of DMA.
            xt_half = []
            t0_0, tb_0 = halves[0][0]
            first_xt = []
            for k in range(KB):
                xtk = xp.tile([PB, tb_0], fp32r, tag=f"xt{k}_0", name=f"xt{k}_0")
                nc.sync.dma_start(
                    out=xtk[:], in_=xT[k * PB:(k + 1) * PB, t0_0:t0_0 + tb_0])
                first_xt.append(xtk)
                wgk = w1q0.tile([PB, QW], fp32r, name=f"wg{k}_0")
                nc.sync.dma_start(out=wgk[:], in_=Wg[k * PB:(k + 1) * PB, 0:QW])
                wg_s[k][0] = wgk
            for k in range(KB):
                wuk = w1q0.tile([PB, QW], fp32r, name=f"wu{k}_0")
                nc.sync.dma_start(out=wuk[:], in_=Wu[k * PB:(k + 1) * PB, 0:QW])
                wu_s[k][0] = wuk
            # Rest of half-0 x tiles, then remaining weight quarters.
            xt_half.append([first_xt] + [load_xt(si, t0, tb)
                                         for si, (t0, tb)
                                         in enumerate(halves[0][1:], start=1)])
            for q in range(1, NQ):
                for k in range(KB):
                    wgk = w1.tile([PB, QW], fp32r, name=f"wg{k}_{q}")
                    nc.sync.dma_start(
                        out=wgk[:], in_=Wg[k * PB:(k + 1) * PB, q * QW:(q + 1) * QW])
                    wg_s[k][q] = wgk
                for k in range(KB):
                    wuk = w1.tile([PB, QW], fp32r, name=f"wu{k}_{q}")
                    nc.sync.dma_start(
                        out=wuk[:], in_=Wu[k * PB:(k + 1) * PB, q * QW:(q + 1) * QW])
                    wu_s[k][q] = wuk
            # Preload the first Wd tiles during phase 1 (disjoint SBUF).
            wd_early = []
            for ib in range(N_EARLY):
                wdk = wde.tile([PB, H], fp32r, name=f"wde{ib}")
                nc.sync.dma_start(out=wdk[:], in_=Wd[ib * PB:(ib + 1) * PB, :])
                wd_early.append(wdk)

            for hi, half in enumerate(halves):
                if hi > 0:
                    xt_half.append([load_xt(si, t0, tb)
                                    for si, (t0, tb) in enumerate(half)])
                for q in range(NQ):
                    if hi == len(halves) - 1 and q == 1:
                        # q0 is dead everywhere: release its SBUF and use it
                        # to preload 8 more Wd tiles during the phase-1 tail.
                        w1q0_stack.close()
                        w2a = _stack.enter_context(
                            tc.tile_pool(name="w2a", bufs=1, side="right"))
                        for ib in range(N_EARLY, min(IB, N_EARLY + 8)):
                            wdk = w2a.tile([PB, H], fp32r, name=f"wda{ib}")
                            nc.sync.dma_start(
                                out=wdk[:], in_=Wd[ib * PB:(ib + 1) * PB, :])
                            wd_early.append(wdk)
                    for si, (t0, tb) in enumerate(half):
                        xt = xt_half[hi][si]
                        for ii in range(IPQ):
                            ib = q * IPQ + ii
                            qc = ii * PB
                            pg = ps1.tile([PB, tb], fp32, tag="pg", name="pg")
                            pu = ps1.tile([PB, tb], fp32, tag="pu", name="pu")
                            for k in range(KB):
                                nc.tensor.matmul(
                                    pg[:],
                                    wg_s[k][q][:, qc:qc + PB],
                                    xt[k][:],
                                    start=(k == 0), stop=(k == KB - 1))
                            for k in range(KB):
                                nc.tensor.matmul(
                                    pu[:],
                                    wu_s[k][q][:, qc:qc + PB],
                                    xt[k][:],
                                    start=(k == 0), stop=(k == KB - 1))
                            sg = ev1.tile([PB, tb], fp32, tag="sg", name="sg")
                            nc.scalar.activation(sg[:], pg[:], AF.Sigmoid)
                            sx = ev1.tile([PB, tb], fp32, tag="sx", name="sx")
                            nc.vector.tensor_mul(sx[:], sg[:], pg[:])
                            hh = ev1.tile([PB, tb], fp32r, tag="hh", name="hh")
                            nc.vector.tensor_mul(hh[:], sx[:], pu[:])
                            nc.gpsimd.dma_start(
                                out=hsp[ib * PB:(ib + 1) * PB, t0:t0 + tb],
                                in_=hh[:])

        # Phase 2: yT = (h @ Wd) * gate.  ib-outer: all 8 output blocks
        # accumulate in 8 PSUM banks so compute starts after wd0+ht0 land.
        with tc.tile_pool(name="w2", bufs=1) as w2, \
             tc.tile_pool(name="hl", bufs=3) as hl, \
             tc.tile_pool(name="ev2", bufs=8) as ev2, \
             tc.tile_pool(name="ps2", bufs=1, space="PSUM") as ps2:
            def load_ht(t0, tb, ib):
                htk = hl.tile([PB, tb], fp32r, tag=f"ht{ib}", name=f"ht{ib}")
                nc.sync.dma_start(
                    out=htk[:], in_=hsp[ib * PB:(ib + 1) * PB, t0:t0 + tb])
                return htk

            # Interleave remaining wd tiles with seg-0 h tiles in need-order.
            wd_s = list(wd_early)
            ht_next = []   # seg0 tiles
            for ib in range(IB):
                if ib >= len(wd_early):
                    wdk = w2.tile([PB, H], fp32r, name=f"wd{ib}")
                    nc.sync.dma_start(out=wdk[:], in_=Wd[ib * PB:(ib + 1) * PB, :])
                    wd_s.append(wdk)
                ht_next.append(load_ht(segs[0][0], segs[0][1], ib))
            gt = w2.tile([PB, C], fp32, name="gt")
            nc.sync.dma_start(out=gt[:], in_=gm[:])
            for si, (t0, tb) in enumerate(segs):
                ht = ht_next
                # queue the next segment's h tiles
                if si + 1 < len(segs):
                    nt0, ntb = segs[si + 1]
                    ht_next = [load_ht(nt0, ntb, ib) for ib in range(IB)]
                py = [ps2.tile([PB, tb], fp32, tag=f"py{hb}", name=f"py{hb}")
                      for hb in range(HB)]
                for ib in range(IB):
                    last = ib == IB - 1
                    for hb in range(HB):
                        nc.tensor.matmul(
                            py[hb][:],
                            wd_s[ib][:, hb * PB:(hb + 1) * PB],
                            ht[ib][:],
                            start=(ib == 0), stop=last)
                        if last:
                            # evict as soon as this output block finishes;
                            # the final segment flushes on the idle HWDGE
                            # queue (all loads are done by then).
                            yt = ev2.tile([PB, tb], fp32, tag="yt", name="yt")
                            nc.vector.tensor_mul(yt[:], py[hb][:],
                                                 gt[:, t0:t0 + tb])
                            eng = (nc.sync if si == len(segs) - 1
                                   else nc.gpsimd)
                            eng.dma_start(
                                out=yT[hb * PB:(hb + 1) * PB, t0:t0 + tb],
                                in_=yt[:])
    nc.compile()
    return nc


def _route(x, Wr, br):
    """Replicate the reference's fp32 router bit-compatibly on host."""
    logits = x @ Wr + br                       # fp32 GEMM
    order = np.argsort(-logits, axis=1, kind="stable")  # ties -> lowest index
    topk_idx = order[:, :TOPK]
    topk_vals = np.take_along_axis(logits, topk_idx, axis=1)
    g = 1.0 / (1.0 + np.exp(-topk_vals.astype(np.float32)))
    g = g / (np.sum(g, axis=-1, keepdims=True) + 1e-10)
    return topk_idx, g.astype(np.float32)


def kernel(x, Wr, br, Wg, Wu, Wd):
    global last_results
    from concourse.bass_utils import run_bass_kernel_spmd

    x = np.asarray(x, dtype=np.float32)
    Wr = np.asarray(Wr, dtype=np.float32)
    br = np.asarray(br, dtype=np.float32)
    Wg = np.asarray(Wg, dtype=np.float32)
    Wu = np.asarray(Wu, dtype=np.float32)
    Wd = np.asarray(Wd, dtype=np.float32)

    topk_idx, g = _route(x, Wr, br)

    # Per-expert token lists
    idx_lists = []
    gate_lists = []
    for e in range(E):
        mask = topk_idx == e                    # [T, K]
        tok = np.nonzero(mask.any(axis=1))[0]
        # gate value for expert e per selected token (slot 0 or slot 1)
        gsel = np.where(mask[tok, 0], g[tok, 0], g[tok, 1]).astype(np.float32)
        idx_lists.append(tok.astype(np.int64))
        gate_lists.append(gsel)

    counts = [len(ix) for ix in idx_lists]
    C = max(512, max(counts))

    key = C
    if key not in _compiled:
        _compiled[key] = _build(C)
    nc = _compiled[key]

    xTf = round_fp32r(np.ascontiguousarray(x.T))   # [H, T], pre-rounded
    in_maps = []
    for e in range(E):
        n = counts[e]
        xTe = np.zeros((H, C), dtype=np.float32)
        xTe[:, :n] = xTf[:, idx_lists[e]]
        gme = np.zeros((PB, C), dtype=np.float32)
        gme[:, :n] = gate_lists[e][None, :]
        in_maps.append({
            "xT": xTe,
            "gm": gme,
            "Wg": round_fp32r(Wg[e]),
            "Wu": round_fp32r(Wu[e]),
            "Wd": round_fp32r(Wd[e]),
        })

    trace = bool(int(os.environ.get("MOE_TRACE", "0")))
    trace_cores = (list(range(NCORES))
                   if os.environ.get("MOE_TRACE_ALL") else None)
    last_results = run_bass_kernel_spmd(
        nc, in_maps, core_ids=list(range(NCORES)), trace=trace,
        trace_cores=trace_cores)

    out = np.zeros((T, H), dtype=np.float32)
    for e in range(E):
        n = counts[e]
        yTe = last_results.results[e]["yT"]
        out[idx_lists[e]] += yTe[:, :n].T
    return out



# revision 4
# speedup vs baseline: 1.0218x; 1.0218x over previous
"""MoE (top-2 of 8 experts, SwiGLU) Trainium2 kernel — fused bf16 single pass.

Strategy (expert-parallel over 8 NeuronCores):
  * Host: router GEMM + top-2 + sigmoid gates in numpy (selection verified to
    match the jax fp32 reference on these inputs), then gather each expert's
    tokens into a transposed, capacity-padded bf16 buffer xT_e [H, C]. One
    expert per core, capacity C = max_e count_e.
  * Device (SPMD, per core): all three weight matrices live in SBUF as bf16
    (12 MB total), x and the gates are SBUF-resident too.  Tokens are
    processed in segments of <=512 (the PSUM free-dim limit); for each
    segment the SwiGLU intermediate h = silu(x@Wg) * (x@Wu) is produced
    i-block by i-block into SBUF (bf16) and immediately consumed by the
    down-projection y = (h@Wd) * gate — h never leaves the chip.
    The down-projection runs hb-outer so only 2 PSUM banks are needed and
    eviction is progressive (short kernel tail).
  * bf16 matmuls stream at the same 1 cycle/row as fp32r but halve SBUF and
    HBM traffic and enable fast weight loads; accumulation is fp32 in PSUM.
    The measured output error vs the fp64 reference is ~3e-3.
  * A burst of warm-up matmuls on the first x tile runs while weights are
    still streaming in, so the PE's HAM clock gate reaches full rate before
    the real matmul stream starts.
  * Host: out[idx_e] += yT_e[:, :n_e].T  (indices within one expert are
    unique, so fancy-index += is safe).
"""

import os
import numpy as np
import ml_dtypes

T, H, I, E, TOPK = 8192, 1024, 2048, 8, 2
NCORES = 8
PB = 128
KB = H // PB   # 8 contraction blocks over H
IB = I // PB   # 16 blocks over I
HB = H // PB   # 8 output blocks over H

W0 = 256       # first Wg/Wu column chunk (i-blocks 0-1) for a fast start
C1 = 1024      # second chunk boundary: c1 = [W0:C1] covers i-blocks 2-7

_compiled = {}
last_results = None  # BassKernelResults of the most recent run (for test harness)


def _tsegs(C):
    """Split C into segments of width 256..512."""
    widths = []
    rem = C
    while rem >= 768:
        widths.append(512)
        rem -= 512
    if rem <= 512:
        widths.append(rem)
    else:
        widths.append(rem - 256)
        widths.append(256)
    segs = []
    t0 = 0
    for tb in widths:
        segs.append((t0, tb))
        t0 += tb
    return segs


def _build(C):
    import concourse.bacc as bacc
    import concourse.mybir as mybir
    import concourse.tile as tile
    from contextlib import ExitStack

    fp32 = mybir.dt.float32
    bf16 = mybir.dt.bfloat16
    AF = mybir.ActivationFunctionType

    nc = bacc.Bacc("TRN2", target_bir_lowering=False, debug=False,
                   num_devices=NCORES)
    xT = nc.dram_tensor("xT", [H, C], bf16, kind="ExternalInput").ap()
    gm = nc.dram_tensor("gm", [PB, C], fp32, kind="ExternalInput").ap()
    Wg = nc.dram_tensor("Wg", [H, I], bf16, kind="ExternalInput").ap()
    Wu = nc.dram_tensor("Wu", [H, I], bf16, kind="ExternalInput").ap()
    Wd = nc.dram_tensor("Wd", [I, H], bf16, kind="ExternalInput").ap()
    yT = nc.dram_tensor("yT", [H, C], fp32, kind="ExternalOutput").ap()

    segs = _tsegs(C)
    s0w = segs[0][1]

    with tile.TileContext(nc) as tc, ExitStack() as st:
        wp = st.enter_context(tc.tile_pool(name="wp", bufs=1))
        hp = st.enter_context(tc.tile_pool(name="hp", bufs=2))
        ev1 = st.enter_context(tc.tile_pool(name="ev1", bufs=2))
        ev2 = st.enter_context(tc.tile_pool(name="ev2", bufs=3))
        ps1 = st.enter_context(tc.tile_pool(name="ps1", bufs=2, space="PSUM"))
        ps2 = st.enter_context(tc.tile_pool(name="ps2", bufs=2, space="PSUM"))
        psw = st.enter_context(tc.tile_pool(name="psw", bufs=1, space="PSUM"))

        # ---- load issue order (three queues so issue overhead ~650ns/DMA
        # doesn't serialize the critical first tiles) ----
        xq = [nc.sync, nc.gpsimd, nc.scalar]
        xs0 = []
        for k in range(KB):
            t = wp.tile([PB, s0w], bf16, name=f"xs0_{k}")
            xq[k % 3].dma_start(out=t[:], in_=xT[k * PB:(k + 1) * PB, 0:s0w])
            xs0.append(t)

        # Warm-up matmuls on the first x tile: keep the PE busy from ~0.7us
        # so the HAM clock gate is at 8/8 before the real stream starts.
        pwarm = psw.tile([PB, s0w], fp32, name="pwarm")
        for _ in range(10):
            nc.tensor.matmul(pwarm[:], xs0[0][:, 0:PB], xs0[0][:],
                             start=True, stop=True)

        wga, wua = [], []
        for k in range(KB):
            t = wp.tile([PB, W0], bf16, name=f"wga{k}")
            nc.sync.dma_start(out=t[:], in_=Wg[k * PB:(k + 1) * PB, 0:W0])
            wga.append(t)
            t = wp.tile([PB, W0], bf16, name=f"wua{k}")
            nc.gpsimd.dma_start(out=t[:], in_=Wu[k * PB:(k + 1) * PB, 0:W0])
            wua.append(t)
        wgb, wub, wgc, wuc = [], [], [], []
        for k in range(KB):
            t = wp.tile([PB, C1 - W0], bf16, name=f"wgb{k}")
            nc.sync.dma_start(out=t[:], in_=Wg[k * PB:(k + 1) * PB, W0:C1])
            wgb.append(t)
            t = wp.tile([PB, C1 - W0], bf16, name=f"wub{k}")
            nc.gpsimd.dma_start(out=t[:], in_=Wu[k * PB:(k + 1) * PB, W0:C1])
            wub.append(t)
        for k in range(KB):
            t = wp.tile([PB, I - C1], bf16, name=f"wgc{k}")
            nc.sync.dma_start(out=t[:], in_=Wg[k * PB:(k + 1) * PB, C1:I])
            wgc.append(t)
            t = wp.tile([PB, I - C1], bf16, name=f"wuc{k}")
            nc.gpsimd.dma_start(out=t[:], in_=Wu[k * PB:(k + 1) * PB, C1:I])
            wuc.append(t)
        gt = wp.tile([PB, C], fp32, name="gt")
        nc.gpsimd.dma_start(out=gt[:], in_=gm[:])

        # Wd tiles and the x remainder are allocated now but DMA'd lazily,
        # interleaved into segment-0 compute (vector/scalar are idle then).
        wd = [wp.tile([PB, H], bf16, name=f"wd{ib}") for ib in range(IB)]
        xr = [wp.tile([PB, C - s0w], bf16, name=f"xr{k}") for k in range(KB)] \
            if C > s0w else []

        def wg_sl(k, ib):
            c = ib * PB
            if c < W0:
                return wga[k][:, c:c + PB]
            if c < C1:
                return wgb[k][:, c - W0:c - W0 + PB]
            return wgc[k][:, c - C1:c - C1 + PB]

        def wu_sl(k, ib):
            c = ib * PB
            if c < W0:
                return wua[k][:, c:c + PB]
            if c < C1:
                return wub[k][:, c - W0:c - W0 + PB]
            return wuc[k][:, c - C1:c - C1 + PB]

        def x_sl(k, t0, w):
            if t0 >= s0w:
                return xr[k][:, t0 - s0w:t0 - s0w + w]
            return xs0[k][:, t0:t0 + w]

        for si, (t0, w) in enumerate(segs):
            last_seg = si == len(segs) - 1
            hts = []
            for ib in range(IB):
                pg = ps1.tile([PB, w], fp32, tag="pg", name="pg")
                pu = ps1.tile([PB, w], fp32, tag="pu", name="pu")
                for k in range(KB):
                    nc.tensor.matmul(pg[:], wg_sl(k, ib), x_sl(k, t0, w),
                                     start=(k == 0), stop=(k == KB - 1))
                for k in range(KB):
                    nc.tensor.matmul(pu[:], wu_sl(k, ib), x_sl(k, t0, w),
                                     start=(k == 0), stop=(k == KB - 1))
                sg = ev1.tile([PB, w], fp32, tag="sg", name="sg")
                nc.scalar.activation(sg[:], pg[:], AF.Silu)
                hh = hp.tile([PB, w], bf16, tag=f"h{ib}", name=f"h{ib}")
                nc.vector.tensor_mul(hh[:], sg[:], pu[:])
                hts.append(hh)
                if si == 0:
                    # trickle the phase-2 loads into phase-1 compute
                    nc.scalar.dma_start(
                        out=wd[ib][:], in_=Wd[ib * PB:(ib + 1) * PB, :])
                    if xr and ib % 2 == 0:
                        k = ib // 2
                        nc.gpsimd.dma_start(
                            out=xr[k][:], in_=xT[k * PB:(k + 1) * PB, s0w:C])
            for hb in range(HB):
                py = ps2.tile([PB, w], fp32, tag="py", name="py")
                for ib in range(IB):
                    nc.tensor.matmul(py[:], wd[ib][:, hb * PB:(hb + 1) * PB],
                                     hts[ib][:],
                                     start=(ib == 0), stop=(ib == IB - 1))
                yt = ev2.tile([PB, w], fp32, tag="yt", name="yt")
                nc.vector.tensor_mul(yt[:], py[:], gt[:, t0:t0 + w])
                eng = nc.sync if last_seg else nc.gpsimd
                eng.dma_start(out=yT[hb * PB:(hb + 1) * PB, t0:t0 + w],
                              in_=yt[:])
    nc.compile()
    return nc


def _route(x, Wr, br):
    """Replicate the reference's fp32 router bit-compatibly on host."""
    logits = x @ Wr + br                       # fp32 GEMM
    order = np.argsort(-logits, axis=1, kind="stable")  # ties -> lowest index
    topk_idx = order[:, :TOPK]
    topk_vals = np.take_along_axis(logits, topk_idx, axis=1)
    g = 1.0 / (1.0 + np.exp(-topk_vals.astype(np.float32)))
    g = g / (np.sum(g, axis=-1, keepdims=True) + 1e-10)
    return topk_idx, g.astype(np.float32)


def kernel(x, Wr, br, Wg, Wu, Wd):
    global last_results
    from concourse.bass_utils import run_bass_kernel_spmd

    x = np.asarray(x, dtype=np.float32)
    Wr = np.asarray(Wr, dtype=np.float32)
    br = np.asarray(br, dtype=np.float32)
    Wg = np.asarray(Wg, dtype=np.float32)
    Wu = np.asarray(Wu, dtype=np.float32)
    Wd = np.asarray(Wd, dtype=np.float32)

    topk_idx, g = _route(x, Wr, br)

    # Per-expert token lists
    idx_lists = []
    gate_lists = []
    for e in range(E):
        mask = topk_idx == e                    # [T, K]
        tok = np.nonzero(mask.any(axis=1))[0]
        gsel = np.where(mask[tok, 0], g[tok, 0], g[tok, 1]).astype(np.float32)
        idx_lists.append(tok.astype(np.int64))
        gate_lists.append(gsel)

    counts = [len(ix) for ix in idx_lists]
    C = max(512, max(counts))

    key = C
    if key not in _compiled:
        _compiled[key] = _build(C)
    nc = _compiled[key]

    bf16 = ml_dtypes.bfloat16
    xTb = np.ascontiguousarray(x.T).astype(bf16)   # [H, T]
    in_maps = []
    for e in range(E):
        n = counts[e]
        xTe = np.zeros((H, C), dtype=bf16)
        xTe[:, :n] = xTb[:, idx_lists[e]]
        gme = np.zeros((PB, C), dtype=np.float32)
        gme[:, :n] = gate_lists[e][None, :]
        in_maps.append({
            "xT": xTe,
            "gm": gme,
            "Wg": Wg[e].astype(bf16),
            "Wu": Wu[e].astype(bf16),
            "Wd": Wd[e].astype(bf16),
        })

    trace = bool(int(os.environ.get("MOE_TRACE", "0")))
    trace_cores = (list(range(NCORES))
                   if os.environ.get("MOE_TRACE_ALL") else None)
    last_results = run_bass_kernel_spmd(
        nc, in_maps, core_ids=list(range(NCORES)), trace=trace,
        trace_cores=trace_cores)

    out = np.zeros((T, H), dtype=np.float32)
    for e in range(E):
        n = counts[e]
        yTe = last_results.results[e]["yT"]
        out[idx_lists[e]] += yTe[:, :n].T
    return out


# revision 8
# speedup vs baseline: 1.0518x; 1.0293x over previous
"""MoE (top-2 of 8 experts, SwiGLU) Trainium2 kernel — fused bf16 single pass.

Strategy (expert-parallel over 8 NeuronCores):
  * Host: router GEMM + top-2 + sigmoid gates in numpy (selection verified to
    match the jax fp32 reference on these inputs), then gather each expert's
    tokens into a transposed, capacity-padded bf16 buffer xT_e [H, C]. One
    expert per core, capacity C = max_e count_e.
  * Device (SPMD, per core): all three weight matrices live in SBUF as bf16
    (12 MB total), x and the gates are SBUF-resident too.  Tokens are
    processed in segments of <=512 (the PSUM free-dim limit); for each
    segment the SwiGLU intermediate h = silu(x@Wg) * (x@Wu) is produced
    i-block by i-block into SBUF (bf16) and immediately consumed by the
    down-projection y = (h@Wd) * gate — h never leaves the chip.
    The down-projection runs hb-outer so only 2 PSUM banks are needed and
    eviction is progressive (short kernel tail).
  * bf16 matmuls stream at the same 1 cycle/row as fp32r but halve SBUF and
    HBM traffic and enable fast weight loads; accumulation is fp32 in PSUM.
    The measured output error vs the fp64 reference is ~3e-3.
  * A burst of warm-up matmuls on the first x tile runs while weights are
    still streaming in, so the PE's HAM clock gate reaches full rate before
    the real matmul stream starts.
  * Host: out[idx_e] += yT_e[:, :n_e].T  (indices within one expert are
    unique, so fancy-index += is safe).
"""

import os
import numpy as np
import ml_dtypes

T, H, I, E, TOPK = 8192, 1024, 2048, 8, 2
NCORES = 8
PB = 128
KB = H // PB   # 8 contraction blocks over H
IB = I // PB   # 16 blocks over I
HB = H // PB   # 8 output blocks over H

# Wg/Wu column chunks (need-ordered streaming): covers i-blocks 0-1, 2-7, 8-15
WCH = [(0, 256), (256, 1024), (1024, 2048)]

_compiled = {}
last_results = None  # BassKernelResults of the most recent run (for test harness)


def _tsegs(C):
    """Split C into segments of width 256..512."""
    widths = []
    rem = C
    while rem >= 768:
        widths.append(512)
        rem -= 512
    if rem <= 512:
        widths.append(rem)
    else:
        widths.append(rem - 256)
        widths.append(256)
    segs = []
    t0 = 0
    for tb in widths:
        segs.append((t0, tb))
        t0 += tb
    return segs


def _build(C):
    import concourse.bacc as bacc
    import concourse.mybir as mybir
    import concourse.tile as tile
    from contextlib import ExitStack

    fp32 = mybir.dt.float32
    bf16 = mybir.dt.bfloat16
    AF = mybir.ActivationFunctionType

    nc = bacc.Bacc("TRN2", target_bir_lowering=False, debug=False,
                   num_devices=NCORES)
    xT = nc.dram_tensor("xT", [H, C], bf16, kind="ExternalInput").ap()
    gm = nc.dram_tensor("gm", [PB, C], fp32, kind="ExternalInput").ap()
    Wg = nc.dram_tensor("Wg", [H, I], bf16, kind="ExternalInput").ap()
    Wu = nc.dram_tensor("Wu", [H, I], bf16, kind="ExternalInput").ap()
    Wd = nc.dram_tensor("Wd", [I, H], bf16, kind="ExternalInput").ap()
    yT = nc.dram_tensor("yT", [H, C], fp32, kind="ExternalOutput").ap()

    segs = _tsegs(C)
    s0w = segs[0][1]

    with tile.TileContext(nc) as tc, ExitStack() as st:
        wp = st.enter_context(tc.tile_pool(name="wp", bufs=1))
        hp = st.enter_context(tc.tile_pool(name="hp", bufs=2))
        ev1 = st.enter_context(tc.tile_pool(name="ev1", bufs=2))
        ev2 = st.enter_context(tc.tile_pool(name="ev2", bufs=3))
        ps1 = st.enter_context(tc.tile_pool(name="ps1", bufs=2, space="PSUM"))
        ps2 = st.enter_context(tc.tile_pool(name="ps2", bufs=4, space="PSUM"))

        # ---- load issue order.  The critical stream (x seg0, then Wg/Wu in
        # i-block need-order) is split between the sync and gpsimd queues;
        # everything needed later (gates, Wd, x remainder) goes on the scalar
        # queue, paced behind the per-i-block silu ops so it cannot steal
        # bandwidth from the critical window. ----
        xq = [nc.sync, nc.gpsimd, nc.scalar]
        xs0 = []
        for k in range(KB):
            t = wp.tile([PB, s0w], bf16, name=f"xs0_{k}")
            xq[k % 3].dma_start(out=t[:], in_=xT[k * PB:(k + 1) * PB, 0:s0w])
            xs0.append(t)

        # Warm-up matmuls on the first x tile: keep the PE busy from ~9us
        # (first DMA landing) so the HAM clock gate reaches 8/8 before the
        # real stream starts.  They write rotating ps2 slots, long retired
        # before phase 2 reuses them.
        for i in range(4):
            pwarm = ps2.tile([PB, s0w], fp32, tag="py", name="py")
            nc.tensor.matmul(pwarm[:], xs0[0][:, 0:PB], xs0[0][:],
                             start=True, stop=True)

        wgt = [[None] * len(WCH) for _ in range(KB)]
        wut = [[None] * len(WCH) for _ in range(KB)]
        for c, (c0, c1) in enumerate(WCH):
            for k in range(KB):
                qa, qb = (nc.sync, nc.gpsimd) if (k + c) % 2 == 0 \
                    else (nc.gpsimd, nc.sync)
                t = wp.tile([PB, c1 - c0], bf16, name=f"wg{c}_{k}")
                qa.dma_start(out=t[:], in_=Wg[k * PB:(k + 1) * PB, c0:c1])
                wgt[k][c] = t
                t = wp.tile([PB, c1 - c0], bf16, name=f"wu{c}_{k}")
                qb.dma_start(out=t[:], in_=Wu[k * PB:(k + 1) * PB, c0:c1])
                wut[k][c] = t

        # Late loads: allocated now, DMA'd from the scalar queue inside the
        # segment-0 loop (after silu ops, so they are naturally paced).
        gt = wp.tile([PB, C], fp32, name="gt")
        wd = [wp.tile([PB, H], bf16, name=f"wd{ib}") for ib in range(IB)]
        xr = [wp.tile([PB, C - s0w], bf16, name=f"xr{k}") for k in range(KB)] \
            if C > s0w else []
        late = [(gt, gm[:])]
        late += [(wd[ib], Wd[ib * PB:(ib + 1) * PB, :]) for ib in range(IB)]
        late += [(xr[k], xT[k * PB:(k + 1) * PB, s0w:C]) for k in range(len(xr))]

        def _chunk(ib):
            c = ib * PB
            for j, (c0, c1) in enumerate(WCH):
                if c < c1:
                    return j, c - c0
            raise AssertionError

        def wg_sl(k, ib):
            j, off = _chunk(ib)
            return wgt[k][j][:, off:off + PB]

        def wu_sl(k, ib):
            j, off = _chunk(ib)
            return wut[k][j][:, off:off + PB]

        def x_sl(k, t0, w):
            if t0 >= s0w:
                return xr[k][:, t0 - s0w:t0 - s0w + w]
            return xs0[k][:, t0:t0 + w]

        for si, (t0, w) in enumerate(segs):
            last_seg = si == len(segs) - 1
            hts = []
            for ib in range(IB):
                pg = ps1.tile([PB, w], fp32, tag="pg", name="pg")
                pu = ps1.tile([PB, w], fp32, tag="pu", name="pu")
                for k in range(KB):
                    nc.tensor.matmul(pg[:], wg_sl(k, ib), x_sl(k, t0, w),
                                     start=(k == 0), stop=(k == KB - 1))
                for k in range(KB):
                    nc.tensor.matmul(pu[:], wu_sl(k, ib), x_sl(k, t0, w),
                                     start=(k == 0), stop=(k == KB - 1))
                sg = ev1.tile([PB, w], fp32, tag="sg", name="sg")
                nc.scalar.activation(sg[:], pg[:], AF.Silu)
                hh = hp.tile([PB, w], bf16, tag=f"h{ib}", name=f"h{ib}")
                nc.vector.tensor_mul(hh[:], sg[:], pu[:])
                hts.append(hh)
                if si == 0 and ib >= 4:
                    # trickle the late loads (gates, Wd, x remainder) on the
                    # scalar queue, ~2-3 per i-block
                    n_ib = IB - 4
                    lo = (ib - 4) * len(late) // n_ib
                    hi = (ib - 3) * len(late) // n_ib
                    for tdst, tsrc in late[lo:hi]:
                        nc.scalar.dma_start(out=tdst[:], in_=tsrc)
            for hb in range(HB):
                py = ps2.tile([PB, w], fp32, tag="py", name="py")
                for ib in range(IB):
                    nc.tensor.matmul(py[:], wd[ib][:, hb * PB:(hb + 1) * PB],
                                     hts[ib][:],
                                     start=(ib == 0), stop=(ib == IB - 1))
                yt = ev2.tile([PB, w], fp32, tag="yt", name="yt")
                nc.vector.tensor_mul(yt[:], py[:], gt[:, t0:t0 + w])
                eng = nc.sync if last_seg else nc.gpsimd
                eng.dma_start(out=yT[hb * PB:(hb + 1) * PB, t0:t0 + w],
                              in_=yt[:])
    nc.compile()
    return nc


def _route(x, Wr, br):
    """Replicate the reference's fp32 router bit-compatibly on host."""
    logits = x @ Wr + br                       # fp32 GEMM
    order = np.argsort(-logits, axis=1, kind="stable")  # ties -> lowest index
    topk_idx = order[:, :TOPK]
    topk_vals = np.take_along_axis(logits, topk_idx, axis=1)
    g = 1.0 / (1.0 + np.exp(-topk_vals.astype(np.float32)))
    g = g / (np.sum(g, axis=-1, keepdims=True) + 1e-10)
    return topk_idx, g.astype(np.float32)


def kernel(x, Wr, br, Wg, Wu, Wd):
    global last_results
    from concourse.bass_utils import run_bass_kernel_spmd

    x = np.asarray(x, dtype=np.float32)
    Wr = np.asarray(Wr, dtype=np.float32)
    br = np.asarray(br, dtype=np.float32)
    Wg = np.asarray(Wg, dtype=np.float32)
    Wu = np.asarray(Wu, dtype=np.float32)
    Wd = np.asarray(Wd, dtype=np.float32)

    topk_idx, g = _route(x, Wr, br)

    # Per-expert token lists
    idx_lists = []
    gate_lists = []
    for e in range(E):
        mask = topk_idx == e                    # [T, K]
        tok = np.nonzero(mask.any(axis=1))[0]
        gsel = np.where(mask[tok, 0], g[tok, 0], g[tok, 1]).astype(np.float32)
        idx_lists.append(tok.astype(np.int64))
        gate_lists.append(gsel)

    counts = [len(ix) for ix in idx_lists]
    C = max(512, max(counts))

    key = C
    if key not in _compiled:
        _compiled[key] = _build(C)
    nc = _compiled[key]

    bf16 = ml_dtypes.bfloat16
    xTb = np.ascontiguousarray(x.T).astype(bf16)   # [H, T]
    in_maps = []
    for e in range(E):
        n = counts[e]
        xTe = np.zeros((H, C), dtype=bf16)
        xTe[:, :n] = xTb[:, idx_lists[e]]
        gme = np.zeros((PB, C), dtype=np.float32)
        gme[:, :n] = gate_lists[e][None, :]
        in_maps.append({
            "xT": xTe,
            "gm": gme,
            "Wg": Wg[e].astype(bf16),
            "Wu": Wu[e].astype(bf16),
            "Wd": Wd[e].astype(bf16),
        })

    trace = bool(int(os.environ.get("MOE_TRACE", "0")))
    trace_cores = (list(range(NCORES))
                   if os.environ.get("MOE_TRACE_ALL") else None)
    last_results = run_bass_kernel_spmd(
        nc, in_maps, core_ids=list(range(NCORES)), trace=trace,
        trace_cores=trace_cores)

    out = np.zeros((T, H), dtype=np.float32)
    for e in range(E):
        n = counts[e]
        yTe = last_results.results[e]["yT"]
        out[idx_lists[e]] += yTe[:, :n].T
    return out


# revision 13
# speedup vs baseline: 1.0773x; 1.0242x over previous
"""MoE (top-2 of 8 experts, SwiGLU) Trainium2 kernel — fused bf16 single pass.

Strategy (expert-parallel over 8 NeuronCores):
  * Host: router GEMM + top-2 + sigmoid gates in numpy (selection verified to
    match the jax fp32 reference on these inputs), then gather each expert's
    tokens into a transposed, capacity-padded bf16 buffer xT_e [H, C]. One
    expert per core, capacity C = max_e count_e.
  * Device (SPMD, per core): all three weight matrices live in SBUF as bf16
    (12 MB total), x and the gates are SBUF-resident too.  Tokens are
    processed in segments of <=512 (the PSUM free-dim limit); for each
    segment the SwiGLU intermediate h = silu(x@Wg) * (x@Wu) is produced
    i-block by i-block into SBUF (bf16) and immediately consumed by the
    down-projection y = (h@Wd) * gate — h never leaves the chip.
    The down-projection runs hb-outer so only 2 PSUM banks are needed and
    eviction is progressive (short kernel tail).
  * bf16 matmuls stream at the same 1 cycle/row as fp32r but halve SBUF and
    HBM traffic and enable fast weight loads; accumulation is fp32 in PSUM.
    The measured output error vs the fp64 reference is ~3e-3.
  * A burst of warm-up matmuls on the first x tile runs while weights are
    still streaming in, so the PE's HAM clock gate reaches full rate before
    the real matmul stream starts.
  * Host: out[idx_e] += yT_e[:, :n_e].T  (indices within one expert are
    unique, so fancy-index += is safe).
"""

import os
import numpy as np
import ml_dtypes

T, H, I, E, TOPK = 8192, 1024, 2048, 8, 2
NCORES = 8
PB = 128
KB = H // PB   # 8 contraction blocks over H
IB = I // PB   # 16 blocks over I
HB = H // PB   # 8 output blocks over H

# Wg/Wu i-block chunks (need-ordered streaming, one DMA per chunk).
# Host re-lays Wg/Wu as WgR[r, (ib*KB + k)*PB + c] = Wg[k*PB + r, ib*PB + c]
# so that any i-block range for ALL k-blocks is one contiguous DMA.
WCH = [(0, 1), (1, 2), (2, 4), (4, 8), (8, 12), (12, 16)]
DCH = [(0, 8), (8, 16)]   # Wd i-block chunks (WdR layout, see below)

_compiled = {}
last_results = None  # BassKernelResults of the most recent run (for test harness)


def _tsegs(C):
    """Split C into segments of width 256..512."""
    widths = []
    rem = C
    while rem >= 768:
        widths.append(512)
        rem -= 512
    if rem <= 512:
        widths.append(rem)
    else:
        widths.append(rem - 256)
        widths.append(256)
    segs = []
    t0 = 0
    for tb in widths:
        segs.append((t0, tb))
        t0 += tb
    return segs


def _build(C):
    import concourse.bacc as bacc
    import concourse.mybir as mybir
    import concourse.tile as tile
    from contextlib import ExitStack

    fp32 = mybir.dt.float32
    bf16 = mybir.dt.bfloat16
    AF = mybir.ActivationFunctionType

    nc = bacc.Bacc("TRN2", target_bir_lowering=False, debug=False,
                   num_devices=NCORES)
    xT = nc.dram_tensor("xT", [H, C], bf16, kind="ExternalInput").ap()
    gm = nc.dram_tensor("gm", [PB, C], fp32, kind="ExternalInput").ap()
    Wg = nc.dram_tensor("Wg", [PB, IB * KB * PB], bf16,
                        kind="ExternalInput").ap()
    Wu = nc.dram_tensor("Wu", [PB, IB * KB * PB], bf16,
                        kind="ExternalInput").ap()
    Wd = nc.dram_tensor("Wd", [PB, IB * H], bf16, kind="ExternalInput").ap()
    yT = nc.dram_tensor("yT", [H, C], fp32, kind="ExternalOutput").ap()

    segs = _tsegs(C)
    s0w = segs[0][1]

    with tile.TileContext(nc) as tc, ExitStack() as st:
        wp = st.enter_context(tc.tile_pool(name="wp", bufs=1))
        hp = st.enter_context(tc.tile_pool(name="hp", bufs=2))
        ev1 = st.enter_context(tc.tile_pool(name="ev1", bufs=2))
        ev2 = st.enter_context(tc.tile_pool(name="ev2", bufs=3))
        ps1 = st.enter_context(tc.tile_pool(name="ps1", bufs=2, space="PSUM"))
        ps2 = st.enter_context(tc.tile_pool(name="ps2", bufs=4, space="PSUM"))

        # ---- load issue order.  The critical stream (x seg0, then Wg/Wu in
        # i-block need-order) is split between the sync and gpsimd queues;
        # everything needed later (gates, Wd, x remainder) goes on the scalar
        # queue, paced behind the per-i-block silu ops so it cannot steal
        # bandwidth from the critical window. ----
        xq = [nc.sync, nc.gpsimd, nc.scalar]
        xs0 = []
        for k in range(KB):
            t = wp.tile([PB, s0w], bf16, name=f"xs0_{k}")
            xq[k % 3].dma_start(out=t[:], in_=xT[k * PB:(k + 1) * PB, 0:s0w])
            xs0.append(t)

        # Warm-up matmuls on the first x tile: keep the PE busy from ~9us
        # (first DMA landing) so the HAM clock gate reaches 8/8 before the
        # real stream starts.  They write rotating ps2 slots, long retired
        # before phase 2 reuses them.
        for i in range(4):
            pwarm = ps2.tile([PB, s0w], fp32, tag="py", name="py")
            nc.tensor.matmul(pwarm[:], xs0[0][:, 0:PB], xs0[0][:],
                             start=True, stop=True)

        IBW = KB * PB   # column span of one i-block in the WgR/WuR layout
        wgt, wut = [], []
        for c, (a, b) in enumerate(WCH):
            qa, qb = (nc.sync, nc.gpsimd) if c % 2 == 0 \
                else (nc.gpsimd, nc.sync)
            t = wp.tile([PB, (b - a) * IBW], bf16, name=f"wg{c}")
            qa.dma_start(out=t[:], in_=Wg[:, a * IBW:b * IBW])
            wgt.append(t)
            t = wp.tile([PB, (b - a) * IBW], bf16, name=f"wu{c}")
            qb.dma_start(out=t[:], in_=Wu[:, a * IBW:b * IBW])
            wut.append(t)

        # Late loads: allocated now, DMA'd from the scalar queue inside the
        # segment-0 loop (after silu ops, so they are naturally paced).
        gt = wp.tile([PB, C], fp32, name="gt")
        wdt = [wp.tile([PB, (b - a) * H], bf16, name=f"wd{c}")
               for c, (a, b) in enumerate(DCH)]
        xr = [wp.tile([PB, C - s0w], bf16, name=f"xr{k}") for k in range(KB)] \
            if C > s0w else []
        late = [(gt, gm[:])]
        late += [(wdt[c], Wd[:, a * H:b * H]) for c, (a, b) in enumerate(DCH)]
        late += [(xr[k], xT[k * PB:(k + 1) * PB, s0w:C]) for k in range(len(xr))]

        def _chunk(ch, ib):
            for j, (a, b) in enumerate(ch):
                if ib < b:
                    return j, ib - a
            raise AssertionError

        def wg_sl(k, ib):
            j, off = _chunk(WCH, ib)
            return wgt[j][:, (off * KB + k) * PB:(off * KB + k) * PB + PB]

        def wu_sl(k, ib):
            j, off = _chunk(WCH, ib)
            return wut[j][:, (off * KB + k) * PB:(off * KB + k) * PB + PB]

        def wd_sl(ib, hb):
            j, off = _chunk(DCH, ib)
            return wdt[j][:, off * H + hb * PB:off * H + hb * PB + PB]

        def x_sl(k, t0, w):
            if t0 >= s0w:
                return xr[k][:, t0 - s0w:t0 - s0w + w]
            return xs0[k][:, t0:t0 + w]

        for si, (t0, w) in enumerate(segs):
            last_seg = si == len(segs) - 1
            hts = []
            for ib in range(IB):
                pg = ps1.tile([PB, w], fp32, tag="pg", name="pg")
                pu = ps1.tile([PB, w], fp32, tag="pu", name="pu")
                for k in range(KB):
                    nc.tensor.matmul(pg[:], wg_sl(k, ib), x_sl(k, t0, w),
                                     start=(k == 0), stop=(k == KB - 1))
                for k in range(KB):
                    nc.tensor.matmul(pu[:], wu_sl(k, ib), x_sl(k, t0, w),
                                     start=(k == 0), stop=(k == KB - 1))
                sg = ev1.tile([PB, w], fp32, tag="sg", name="sg")
                nc.scalar.activation(sg[:], pg[:], AF.Silu)
                hh = hp.tile([PB, w], bf16, tag=f"h{ib}", name=f"h{ib}")
                nc.vector.tensor_mul(hh[:], sg[:], pu[:])
                hts.append(hh)
                if si == 0 and ib >= 4:
                    # trickle the late loads (gates, Wd, x remainder) on the
                    # scalar queue, ~2-3 per i-block
                    n_ib = IB - 4
                    lo = (ib - 4) * len(late) // n_ib
                    hi = (ib - 3) * len(late) // n_ib
                    for tdst, tsrc in late[lo:hi]:
                        nc.scalar.dma_start(out=tdst[:], in_=tsrc)
            for hb in range(HB):
                py = ps2.tile([PB, w], fp32, tag="py", name="py")
                for ib in range(IB):
                    nc.tensor.matmul(py[:], wd_sl(ib, hb), hts[ib][:],
                                     start=(ib == 0), stop=(ib == IB - 1))
                yt = ev2.tile([PB, w], fp32, tag="yt", name="yt")
                nc.vector.tensor_mul(yt[:], py[:], gt[:, t0:t0 + w])
                eng = nc.sync if last_seg else nc.gpsimd
                eng.dma_start(out=yT[hb * PB:(hb + 1) * PB, t0:t0 + w],
                              in_=yt[:])
    nc.compile()
    return nc


def _route(x, Wr, br):
    """Replicate the reference's fp32 router bit-compatibly on host."""
    logits = x @ Wr + br                       # fp32 GEMM
    order = np.argsort(-logits, axis=1, kind="stable")  # ties -> lowest index
    topk_idx = order[:, :TOPK]
    topk_vals = np.take_along_axis(logits, topk_idx, axis=1)
    g = 1.0 / (1.0 + np.exp(-topk_vals.astype(np.float32)))
    g = g / (np.sum(g, axis=-1, keepdims=True) + 1e-10)
    return topk_idx, g.astype(np.float32)


def kernel(x, Wr, br, Wg, Wu, Wd):
    global last_results
    from concourse.bass_utils import run_bass_kernel_spmd

    x = np.asarray(x, dtype=np.float32)
    Wr = np.asarray(Wr, dtype=np.float32)
    br = np.asarray(br, dtype=np.float32)
    Wg = np.asarray(Wg, dtype=np.float32)
    Wu = np.asarray(Wu, dtype=np.float32)
    Wd = np.asarray(Wd, dtype=np.float32)

    topk_idx, g = _route(x, Wr, br)

    # Per-expert token lists
    idx_lists = []
    gate_lists = []
    for e in range(E):
        mask = topk_idx == e                    # [T, K]
        tok = np.nonzero(mask.any(axis=1))[0]
        gsel = np.where(mask[tok, 0], g[tok, 0], g[tok, 1]).astype(np.float32)
        idx_lists.append(tok.astype(np.int64))
        gate_lists.append(gsel)

    counts = [len(ix) for ix in idx_lists]
    C = max(512, max(counts))

    key = C
    if key not in _compiled:
        _compiled[key] = _build(C)
    nc = _compiled[key]

    bf16 = ml_dtypes.bfloat16
    xTb = np.ascontiguousarray(x.T).astype(bf16)   # [H, T]

    def _wR(w):
        # [H, I] -> [PB, IB*KB*PB] with col (ib*KB + k)*PB + c
        return np.ascontiguousarray(
            w.reshape(KB, PB, IB, PB).transpose(1, 2, 0, 3).reshape(PB, -1)
        ).astype(bf16)

    def _wdR(w):
        # [I, H] -> [PB, IB*H] with col ib*H + h
        return np.ascontiguousarray(
            w.reshape(IB, PB, H).transpose(1, 0, 2).reshape(PB, -1)
        ).astype(bf16)

    in_maps = []
    for e in range(E):
        n = counts[e]
        xTe = np.zeros((H, C), dtype=bf16)
        xTe[:, :n] = xTb[:, idx_lists[e]]
        gme = np.zeros((PB, C), dtype=np.float32)
        gme[:, :n] = gate_lists[e][None, :]
        in_maps.append({
            "xT": xTe,
            "gm": gme,
            "Wg": _wR(Wg[e]),
            "Wu": _wR(Wu[e]),
            "Wd": _wdR(Wd[e]),
        })

    trace = bool(int(os.environ.get("MOE_TRACE", "0")))
    trace_cores = (list(range(NCORES))
                   if os.environ.get("MOE_TRACE_ALL") else None)
    last_results = run_bass_kernel_spmd(
        nc, in_maps, core_ids=list(range(NCORES)), trace=trace,
        trace_cores=trace_cores)

    out = np.zeros((T, H), dtype=np.float32)
    for e in range(E):
        n = counts[e]
        yTe = last_results.results[e]["yT"]
        out[idx_lists[e]] += yTe[:, :n].T
    return out


# revision 16
# speedup vs baseline: 1.1117x; 1.0319x over previous
"""MoE (top-2 of 8 experts, SwiGLU) Trainium2 kernel — fused bf16 single pass.

Strategy (expert-parallel over 8 NeuronCores):
  * Host: router GEMM + top-2 + sigmoid gates in numpy (selection verified to
    match the jax fp32 reference on these inputs), then gather each expert's
    tokens into a transposed, capacity-padded bf16 buffer xT_e [H, C]. One
    expert per core, capacity C = max_e count_e.
  * Device (SPMD, per core): all three weight matrices live in SBUF as bf16
    (12 MB total), x and the gates are SBUF-resident too.  Tokens are
    processed in segments of <=512 (the PSUM free-dim limit); for each
    segment the SwiGLU intermediate h = silu(x@Wg) * (x@Wu) is produced
    i-block by i-block into SBUF (bf16) and immediately consumed by the
    down-projection y = (h@Wd) * gate — h never leaves the chip.
    The down-projection runs hb-outer so only 2 PSUM banks are needed and
    eviction is progressive (short kernel tail).
  * bf16 matmuls stream at the same 1 cycle/row as fp32r but halve SBUF and
    HBM traffic and enable fast weight loads; accumulation is fp32 in PSUM.
    The measured output error vs the fp64 reference is ~3e-3.
  * A burst of warm-up matmuls on the first x tile runs while weights are
    still streaming in, so the PE's HAM clock gate reaches full rate before
    the real matmul stream starts.
  * Host: out[idx_e] += yT_e[:, :n_e].T  (indices within one expert are
    unique, so fancy-index += is safe).
"""

import os
import numpy as np
import ml_dtypes

T, H, I, E, TOPK = 8192, 1024, 2048, 8, 2
NCORES = 8
PB = 128
KB = H // PB   # 8 contraction blocks over H
IB = I // PB   # 16 blocks over I
HB = H // PB   # 8 output blocks over H

# Wg/Wu i-block chunks (need-ordered streaming, one DMA per chunk).
# Host re-lays Wg/Wu as WgR[r, (ib*KB + k)*PB + c] = Wg[k*PB + r, ib*PB + c]
# so that any i-block range for ALL k-blocks is one contiguous DMA.
WCH = [(0, 1), (1, 2), (2, 4), (4, 8), (8, 12), (12, 16)]
DCH = [(0, 8), (8, 16)]   # Wd i-block chunks (WdR layout, see below)

_compiled = {}
last_results = None  # BassKernelResults of the most recent run (for test harness)


def _tsegs(C):
    """Split C into segments of width 256..512."""
    widths = []
    rem = C
    while rem >= 768:
        widths.append(512)
        rem -= 512
    if rem <= 512:
        widths.append(rem)
    else:
        widths.append(rem - 256)
        widths.append(256)
    segs = []
    t0 = 0
    for tb in widths:
        segs.append((t0, tb))
        t0 += tb
    return segs


def _build(C):
    import concourse.bacc as bacc
    import concourse.mybir as mybir
    import concourse.tile as tile
    from contextlib import ExitStack

    fp32 = mybir.dt.float32
    bf16 = mybir.dt.bfloat16
    AF = mybir.ActivationFunctionType

    nc = bacc.Bacc("TRN2", target_bir_lowering=False, debug=False,
                   num_devices=NCORES)
    xT = nc.dram_tensor("xT", [H, C], bf16, kind="ExternalInput").ap()
    gm = nc.dram_tensor("gm", [PB, C], fp32, kind="ExternalInput").ap()
    Wg = nc.dram_tensor("Wg", [PB, IB * KB * PB], bf16,
                        kind="ExternalInput").ap()
    Wu = nc.dram_tensor("Wu", [PB, IB * KB * PB], bf16,
                        kind="ExternalInput").ap()
    Wd = nc.dram_tensor("Wd", [PB, IB * H], bf16, kind="ExternalInput").ap()
    yT = nc.dram_tensor("yT", [H, C], fp32, kind="ExternalOutput").ap()

    segs = _tsegs(C)
    s0w = segs[0][1]

    with tile.TileContext(nc) as tc, ExitStack() as st:
        wp = st.enter_context(tc.tile_pool(name="wp", bufs=1))
        hp = st.enter_context(tc.tile_pool(name="hp", bufs=2))
        ev1 = st.enter_context(tc.tile_pool(name="ev1", bufs=2))
        ev2 = st.enter_context(tc.tile_pool(name="ev2", bufs=3))
        ps1 = st.enter_context(tc.tile_pool(name="ps1", bufs=2, space="PSUM"))
        ps2 = st.enter_context(tc.tile_pool(name="ps2", bufs=4, space="PSUM"))

        # ---- load issue order.  The critical stream (x seg0, then Wg/Wu in
        # i-block need-order) is split between the sync and gpsimd queues;
        # everything needed later (gates, Wd, x remainder) goes on the scalar
        # queue, paced behind the per-i-block silu ops so it cannot steal
        # bandwidth from the critical window. ----
        xq = [nc.sync, nc.gpsimd, nc.scalar]
        xs0 = []
        for k in range(KB):
            t = wp.tile([PB, s0w], bf16, name=f"xs0_{k}")
            xq[k % 3].dma_start(out=t[:], in_=xT[k * PB:(k + 1) * PB, 0:s0w])
            xs0.append(t)

        # Warm-up matmuls on the first x tile: keep the PE busy from ~9us
        # (first DMA landing) so the HAM clock gate reaches 8/8 before the
        # real stream starts.  They write rotating ps2 slots, long retired
        # before phase 2 reuses them.
        for i in range(3):
            pwarm = ps2.tile([PB, s0w], fp32, tag="py", name="py")
            nc.tensor.matmul(pwarm[:], xs0[0][:, 0:PB], xs0[0][:],
                             start=True, stop=True)

        IBW = KB * PB   # column span of one i-block in the WgR/WuR layout
        wgt, wut = [], []
        for c, (a, b) in enumerate(WCH):
            qa, qb = (nc.sync, nc.gpsimd) if c % 2 == 0 \
                else (nc.gpsimd, nc.sync)
            t = wp.tile([PB, (b - a) * IBW], bf16, name=f"wg{c}")
            qa.dma_start(out=t[:], in_=Wg[:, a * IBW:b * IBW])
            wgt.append(t)
            t = wp.tile([PB, (b - a) * IBW], bf16, name=f"wu{c}")
            qb.dma_start(out=t[:], in_=Wu[:, a * IBW:b * IBW])
            wut.append(t)

        # Late loads (gates, Wd, x remainder): issued at the BACK of the sync
        # and gpsimd queues.  In-queue FIFO ordering paces their transfers
        # behind the critical Wg/Wu stream — the Tile scheduler would hoist
        # them if they sat dep-free on an otherwise-busy engine.
        gt = wp.tile([PB, C], fp32, name="gt")
        wdt = [wp.tile([PB, (b - a) * H], bf16, name=f"wd{c}")
               for c, (a, b) in enumerate(DCH)]
        xr = [wp.tile([PB, C - s0w], bf16, name=f"xr{k}") for k in range(KB)] \
            if C > s0w else []
        nc.gpsimd.dma_start(out=gt[:], in_=gm[:])
        for c, (a, b) in enumerate(DCH):
            q = nc.sync if c % 2 == 0 else nc.gpsimd
            q.dma_start(out=wdt[c][:], in_=Wd[:, a * H:b * H])
        for k in range(len(xr)):
            q = nc.sync if k % 2 == 0 else nc.gpsimd
            q.dma_start(out=xr[k][:], in_=xT[k * PB:(k + 1) * PB, s0w:C])

        def _chunk(ch, ib):
            for j, (a, b) in enumerate(ch):
                if ib < b:
                    return j, ib - a
            raise AssertionError

        def wg_sl(k, ib):
            j, off = _chunk(WCH, ib)
            return wgt[j][:, (off * KB + k) * PB:(off * KB + k) * PB + PB]

        def wu_sl(k, ib):
            j, off = _chunk(WCH, ib)
            return wut[j][:, (off * KB + k) * PB:(off * KB + k) * PB + PB]

        def wd_sl(ib, hb):
            j, off = _chunk(DCH, ib)
            return wdt[j][:, off * H + hb * PB:off * H + hb * PB + PB]

        def x_sl(k, t0, w):
            if t0 >= s0w:
                return xr[k][:, t0 - s0w:t0 - s0w + w]
            return xs0[k][:, t0:t0 + w]

        for si, (t0, w) in enumerate(segs):
            last_seg = si == len(segs) - 1
            hts = []
            for ib in range(IB):
                pg = ps1.tile([PB, w], fp32, tag="pg", name="pg")
                pu = ps1.tile([PB, w], fp32, tag="pu", name="pu")
                for k in range(KB):
                    nc.tensor.matmul(pg[:], wg_sl(k, ib), x_sl(k, t0, w),
                                     start=(k == 0), stop=(k == KB - 1))
                for k in range(KB):
                    nc.tensor.matmul(pu[:], wu_sl(k, ib), x_sl(k, t0, w),
                                     start=(k == 0), stop=(k == KB - 1))
                sg = ev1.tile([PB, w], fp32, tag="sg", name="sg")
                nc.scalar.activation(sg[:], pg[:], AF.Silu)
                hh = hp.tile([PB, w], bf16, tag=f"h{ib}", name=f"h{ib}")
                nc.vector.tensor_mul(hh[:], sg[:], pu[:])
                hts.append(hh)

            for hb in range(HB):
                py = ps2.tile([PB, w], fp32, tag="py", name="py")
                for ib in range(IB):
                    nc.tensor.matmul(py[:], wd_sl(ib, hb), hts[ib][:],
                                     start=(ib == 0), stop=(ib == IB - 1))
                yt = ev2.tile([PB, w], fp32, tag="yt", name="yt")
                nc.vector.tensor_mul(yt[:], py[:], gt[:, t0:t0 + w])
                eng = nc.sync if last_seg else nc.gpsimd
                eng.dma_start(out=yT[hb * PB:(hb + 1) * PB, t0:t0 + w],
                              in_=yt[:])
    nc.compile()
    return nc


def _route(x, Wr, br):
    """Replicate the reference's fp32 router bit-compatibly on host."""
    logits = x @ Wr + br                       # fp32 GEMM
    order = np.argsort(-logits, axis=1, kind="stable")  # ties -> lowest index
    topk_idx = order[:, :TOPK]
    topk_vals = np.take_along_axis(logits, topk_idx, axis=1)
    g = 1.0 / (1.0 + np.exp(-topk_vals.astype(np.float32)))
    g = g / (np.sum(g, axis=-1, keepdims=True) + 1e-10)
    return topk_idx, g.astype(np.float32)


def kernel(x, Wr, br, Wg, Wu, Wd):
    global last_results
    from concourse.bass_utils import run_bass_kernel_spmd

    x = np.asarray(x, dtype=np.float32)
    Wr = np.asarray(Wr, dtype=np.float32)
    br = np.asarray(br, dtype=np.float32)
    Wg = np.asarray(Wg, dtype=np.float32)
    Wu = np.asarray(Wu, dtype=np.float32)
    Wd = np.asarray(Wd, dtype=np.float32)

    topk_idx, g = _route(x, Wr, br)

    # Per-expert token lists
    idx_lists = []
    gate_lists = []
    for e in range(E):
        mask = topk_idx == e                    # [T, K]
        tok = np.nonzero(mask.any(axis=1))[0]
        gsel = np.where(mask[tok, 0], g[tok, 0], g[tok, 1]).astype(np.float32)
        idx_lists.append(tok.astype(np.int64))
        gate_lists.append(gsel)

    counts = [len(ix) for ix in idx_lists]
    C = max(512, max(counts))

    key = C
    if key not in _compiled:
        _compiled[key] = _build(C)
    nc = _compiled[key]

    bf16 = ml_dtypes.bfloat16
    xTb = np.ascontiguousarray(x.T).astype(bf16)   # [H, T]

    def _wR(w):
        # [H, I] -> [PB, IB*KB*PB] with col (ib*KB + k)*PB + c
        return np.ascontiguousarray(
            w.reshape(KB, PB, IB, PB).transpose(1, 2, 0, 3).reshape(PB, -1)
        ).astype(bf16)

    def _wdR(w):
        # [I, H] -> [PB, IB*H] with col ib*H + h
        return np.ascontiguousarray(
            w.reshape(IB, PB, H).transpose(1, 0, 2).reshape(PB, -1)
        ).astype(bf16)

    in_maps = []
    for e in range(E):
        n = counts[e]
        xTe = np.zeros((H, C), dtype=bf16)
        xTe[:, :n] = xTb[:, idx_lists[e]]
        gme = np.zeros((PB, C), dtype=np.float32)
        gme[:, :n] = gate_lists[e][None, :]
        in_maps.append({
            "xT": xTe,
            "gm": gme,
            "Wg": _wR(Wg[e]),
            "Wu": _wR(Wu[e]),
            "Wd": _wdR(Wd[e]),
        })

    trace = bool(int(os.environ.get("MOE_TRACE", "0")))
    trace_cores = (list(range(NCORES))
                   if os.environ.get("MOE_TRACE_ALL") else None)
    last_results = run_bass_kernel_spmd(
        nc, in_maps, core_ids=list(range(NCORES)), trace=trace,
        trace_cores=trace_cores)

    out = np.zeros((T, H), dtype=np.float32)
    for e in range(E):
        n = counts[e]
        yTe = last_results.results[e]["yT"]
        out[idx_lists[e]] += yTe[:, :n].T
    return out
